# revision 19
# baseline (speedup 1.0000x reference)
"""Trainium2 Bass kernel for nn_Encoder_90469191122997 (gnn_message_passing).

Data-parallel over batch B=8: core b owns batch b end-to-end.

v2: 16-bit traffic + SBUF residency.  x is sent to the device as bf16
(host cast), y is returned as bf16 (host upcast); the 2e-2 rel-err gate
leaves ~5x margin.  Per core x_b = [T*C, HW] bf16 = 28.9 MB; 7 of the 9
128-row chunks stay resident in SBUF between the pooling pass and the
residual pass, chunk 8 stays in the rotating buffer, and only chunk 7 is
re-read.  HBM traffic/core: 28.9 (read) + 3.2 (re-read) + 28.9 (write)
= 61 MB vs 173 MB for the fp32 two-pass baseline.

Pooling uses a 112-column grid (28 blocks x 112 = 3136, no tail).  Per
chunk, either:
  - DVE path: 2x2 box-sum on DVE (bf16) -> x2, then 28 PE transposes, or
  - PE path (KNOBS['pe_chunks']): 4 accumulating PE matmuls per block
    (x strided slice ^T @ identity) produce the box-summed transpose
    directly in fp32 PSUM, freeing DVE for the pass-2 residual adds.
Transposed tiles are copied PSUM->SBUF on ACT (KNOBS['copy_eng']) and
contracted against the pre-transposed masks -> node features [18, 192].
The 18-node GCN (softmax adjacency, two linears, message passing) runs
on-chip in fp32 PSUM with bf16 operands.  Pass 2 matmuls outg against
the 56x56 masks and DVE adds the 2x-nearest-upsampled residual into the
resident x tiles (broadcast APs), which are then DMA'd out as y.
"""

import numpy as np
import ml_dtypes

import concourse.bass as bass
import concourse.mybir as mybir
import concourse.tile as tile
from concourse.masks import make_identity

T, B, C, H, W = 6, 8, 192, 112, 112
K = 3
H0, W0 = 56, 56
HW = H * W            # 12544
HW0 = H0 * W0         # 3136
N = T * K             # 18
CH = 96               # c half
NJ = 28               # pooling blocks per chunk (112-col grid, no tail)
JW = 112              # pooling block width
NR = 7                # residual hw0 chunks of 448 per row-chunk
RW = 448              # residual chunk width at 56-res (8 rows of 56)
NCH = T * C // 128    # 9 row-chunks of 128 (t,c) rows each
NSTASH = 6            # chunks 0..5 resident in SBUF; 6,7 re-read; 8 in rot

BF = mybir.dt.bfloat16
F32 = mybir.dt.float32
BF_NP = ml_dtypes.bfloat16


def _spans(r):
    """(t, lo, hi, clo): rows [lo,hi) of chunk r belong to t, starting at
    channel clo.  Chunk boundaries hit t-edges only at offsets 0/64."""
    out = []
    for t in range(T):
        lo = max(128 * r, C * t)
        hi = min(128 * r + 128, C * (t + 1))
        if lo < hi:
            out.append((t, lo - 128 * r, hi - 128 * r, lo - C * t))
    return out


_LAST_CHUNK = {t: (C * (t + 1) - 1) // 128 for t in range(T)}

_MAX_WAITS = 1


def _split_multi_waits(nc):
    """This container's walrus rejects >1 sem wait per instruction ("Too many
    sync wait commands").  Move extra waits onto same-engine NoOps inserted
    immediately before the instruction (per-engine program order preserved)."""
    for bb in nc.main_func.blocks:
        insts = list(bb.instructions)
        if not any(
            i.sync_info and i.sync_info.on_wait
            and len(i.sync_info.on_wait) > _MAX_WAITS
            for i in insts
        ):
            continue
        new = []
        for inst in insts:
            si = inst.sync_info
            if si and si.on_wait and len(si.on_wait) > _MAX_WAITS:
                extra = list(si.on_wait[_MAX_WAITS:])
                del si.on_wait[_MAX_WAITS:]
                while extra:
                    chunk, extra = extra[:_MAX_WAITS], extra[_MAX_WAITS:]
                    nop = mybir.InstNoOp(
                        name=nc.get_next_instruction_name(),
                        engine=inst.engine,
                        bass_nofuse=True,
                        sync_info=mybir.SyncInfo(on_wait=chunk, on_update=[]),
                    )
                    nc.register_instruction(nop, overwrite=True)
                    new.append(nop)
            new.append(inst)
        bb.instructions = new


_orig_drain_and_barrier = tile.TileContext._drain_and_barrier


def _patched_drain_and_barrier(self, tick_clock, wait_clock):
    _orig_drain_and_barrier(self, tick_clock, wait_clock)
    _split_multi_waits(self.nc)


tile.TileContext._drain_and_barrier = _patched_drain_and_barrier


KNOBS = dict(
    pe_chunks=(),          # chunks whose box-sum+transpose runs on PE
    copy_eng='scalar',     # engine for PSUM->SBUF transpose-tile copies
    add_eng='vector',      # engine for pass-2 residual adds
    rcopy_eng='vector',    # engine for pass-2 res PSUM->SBUF copies
    store_eng='scalar',    # engine issuing y store DMAs
    x2T_bufs=4, tr_bufs=2, res_bufs=4, rsb_bufs=1,
)

if __name__ != "__main__":
    import json as _json
    import os as _os
    _ov = _os.environ.get("KERNEL_KNOBS")
    if _ov:
        KNOBS.update(_json.loads(_ov))


def build_nc(reps: int = 1) -> bass.Bass:
    nc = bass.Bass()
    x = nc.dram_tensor("x", [T * C, HW], BF, kind="ExternalInput")
    m56 = nc.dram_tensor("m56", [N, HW0], BF, kind="ExternalInput")
    mTp = nc.dram_tensor("mTp", [JW, T * NJ * K], BF, kind="ExternalInput")
    wembT = nc.dram_tensor("wembT", [C, C], BF, kind="ExternalInput")
    wgcn = nc.dram_tensor("wgcn", [C, C], BF, kind="ExternalInput")
    bb = nc.dram_tensor("bb", [N, C], F32, kind="ExternalInput")
    y = nc.dram_tensor("y", [T * C, HW], BF, kind="ExternalOutput")

    copy_eng = getattr(nc, KNOBS['copy_eng'])
    add_eng = getattr(nc, KNOBS['add_eng'])
    store_eng = getattr(nc, KNOBS['store_eng'])
    rcopy_eng = getattr(nc, KNOBS['rcopy_eng'])

    with tile.TileContext(nc) as tc:
        with (
            tc.tile_pool(name="persist", bufs=1) as pp,
            tc.tile_pool(name="x2Tpool", bufs=KNOBS['x2T_bufs']) as x2Tpool,
            tc.tile_pool(name="smallsb", bufs=2) as ssb,
        ):
            ident = pp.tile([128, 128], BF)
            make_identity(nc, ident)
            mTp_sb = pp.tile([JW, T * NJ * K], BF)
            nc.sync.dma_start(mTp_sb[:], mTp[:])
            m56_sb = pp.tile([N, HW0], BF)
            nc.sync.dma_start(m56_sb[:], m56[:])
            wemb_h = []
            wgcn_h = []
            for hh in range(2):
                wt = pp.tile([CH, C], BF, tag=f"wemb{hh}")
                nc.sync.dma_start(wt[:], wembT[hh * CH:(hh + 1) * CH, :])
                wemb_h.append(wt)
                gt = pp.tile([CH, C], BF, tag=f"wgcn{hh}")
                nc.sync.dma_start(gt[:], wgcn[hh * CH:(hh + 1) * CH, :])
                wgcn_h.append(gt)
            bb_sb = pp.tile([N, C], F32)
            nc.sync.dma_start(bb_sb[:], bb[:])

            # resident x chunks + rotating buffer + box-sum scratch
            st = [
                pp.tile([128, HW], BF, tag=f"stash{i}", name=f"stash{i}")
                for i in range(NSTASH)
            ]
            rot = pp.tile([128, HW], BF, tag="rot", name="rot")
            x2 = pp.tile([128, HW0], BF, tag="x2", name="x2")

            def chunk_buf(r):
                return st[r] if r < NSTASH else rot

            for rep in range(reps):
                nodeT_h = [
                    pp.tile([CH, N], BF, tag=f"nodeT{hh}", name=f"nodeT{hh}")
                    for hh in range(2)
                ]
                outgb = pp.tile([N, C], BF, tag="outgb", name="outgb")
                outg_t = [
                    pp.tile([N, C], BF, tag=f"outg_t{t}", name=f"outg_t{t}")
                    for t in range(T)
                ]
                lhsr = [
                    pp.tile([N, 128], BF, tag=f"lhsr{r}", name=f"lhsr{r}")
                    for r in range(NCH)
                ]

                # ---------------- pass 1: pooling ----------------
                with (
                    tc.tile_pool(name="trfps", bufs=KNOBS['tr_bufs'],
                                 space="PSUM") as trfps,
                    tc.tile_pool(name="trbps", bufs=KNOBS['tr_bufs'],
                                 space="PSUM") as trbps,
                    tc.tile_pool(name="featps", bufs=3, space="PSUM") as fps,
                    tc.tile_pool(name="ntps", bufs=1, space="PSUM") as ntps,
                ):
                    feat_ps = {}

                    def do_pair(r, js, buf, xq):
                        """Produce x2T tile [112, 256] for block pair
                        (2js, 2js+1) of chunk r and run pooling matmuls.
                        Pairing halves the PSUM->SBUF copy count."""
                        jj = (2 * js, 2 * js + 1)
                        if r in KNOBS['pe_chunks']:
                            tr = trfps.tile([JW, 256], F32, tag="trf")
                            for i, j in enumerate(jj):
                                # 4 accumulating matmuls: out = sum_q plane^T
                                for q in range(4):
                                    nc.tensor.matmul(
                                        tr[:, 128 * i:128 * (i + 1)],
                                        xq[:, q, j * JW:(j + 1) * JW],
                                        ident[:],
                                        start=(q == 0), stop=(q == 3),
                                        skip_group_check=True,
                                    )
                        else:
                            tr = trbps.tile([JW, 256], BF, tag="trb")
                            for i, j in enumerate(jj):
                                nc.tensor.transpose(
                                    tr[:, 128 * i:128 * (i + 1)],
                                    x2[:, j * JW:(j + 1) * JW], ident[:],
                                )
                        x2T = x2Tpool.tile([JW, 256], BF, tag="x2T")
                        if hasattr(copy_eng, 'tensor_copy'):
                            copy_eng.tensor_copy(x2T[:], tr[:])
                        else:
                            copy_eng.copy(x2T[:], tr[:])
                        for i, j in enumerate(jj):
                            for (t, lo, hi, clo) in _spans(r):
                                col = (t * NJ + j) * K
                                nc.tensor.matmul(
                                    feat_ps[t][:, clo:clo + (hi - lo)],
                                    mTp_sb[:, col:col + K],
                                    x2T[:, 128 * i + lo:128 * i + hi],
                                    start=(j == 0), stop=(j == NJ - 1),
                                    skip_group_check=True,
                                )

                    for r in range(NCH):
                        buf = chunk_buf(r)
                        nc.sync.dma_start(buf[:], x[128 * r:128 * (r + 1), :])
                        # phase-major layout: buf = [p, (q hw0)] with q the
                        # 2x2 phase (dh, dw); box-sum = 3 step-1 bf16 adds
                        # (DVE 2x packed mode), no strided APs anywhere
                        xq = buf.rearrange("p (q c) -> p q c", q=4)
                        for (t, lo, hi, clo) in _spans(r):
                            if t not in feat_ps:
                                feat_ps[t] = fps.tile(
                                    [K, C], F32, tag="feat_ps",
                                    name=f"featps{t}",
                                )
                        if r in KNOBS['pe_chunks']:
                            for js in range(NJ // 2):
                                do_pair(r, js, buf, xq)
                        else:
                            # box-sum in two halves so PE transposes of the
                            # first half overlap DVE summing the second
                            for hf in range(2):
                                sl = slice(hf * (HW0 // 2),
                                           (hf + 1) * (HW0 // 2))
                                out = x2[:, sl]
                                nc.vector.tensor_add(out, xq[:, 0, sl],
                                                     xq[:, 1, sl])
                                nc.vector.tensor_add(out, out, xq[:, 2, sl])
                                nc.vector.tensor_add(out, out, xq[:, 3, sl])
                                for js in range(hf * (NJ // 4),
                                                (hf + 1) * (NJ // 4)):
                                    do_pair(r, js, buf, xq)
                        for (t, lo, hi, clo) in _spans(r):
                            if _LAST_CHUNK[t] != r:
                                continue
                            feat_sb = ssb.tile([K, C], BF, tag="feat_sb")
                            nc.scalar.mul(feat_sb[:], feat_ps.pop(t)[:],
                                          1.0 / HW)
                            for hh in range(2):
                                ntr = ntps.tile([CH, K], BF, tag="ntr")
                                nc.tensor.transpose(
                                    ntr[:],
                                    feat_sb[:, hh * CH:(hh + 1) * CH],
                                    ident[:K, :K],
                                )
                                nc.any.tensor_copy(
                                    nodeT_h[hh][:, K * t:K * (t + 1)], ntr[:]
                                )

                # ---------------- GCN on [18, 192] ----------------
                with tc.tile_pool(name="gcnps", bufs=1, space="PSUM") as gps:
                    adjL = gps.tile([N, N], F32, tag="adjL")
                    for hh in range(2):
                        nc.tensor.matmul(
                            adjL[:], nodeT_h[hh][:], nodeT_h[hh][:],
                            start=(hh == 0), stop=(hh == 1),
                        )
                    mx = ssb.tile([N, 1], F32, tag="mx")
                    nc.vector.reduce_max(mx[:], adjL[:], axis=mybir.AxisListType.X)
                    nmx = ssb.tile([N, 1], F32, tag="nmx")
                    nc.vector.tensor_scalar_mul(nmx[:], mx[:], -1.0)
                    e_sb = ssb.tile([N, N], F32, tag="e_sb")
                    nc.scalar.activation(
                        e_sb[:], adjL[:], mybir.ActivationFunctionType.Exp,
                        bias=nmx[:], scale=1.0,
                    )
                    s_ = ssb.tile([N, 1], F32, tag="s_")
                    nc.vector.reduce_sum(s_[:], e_sb[:], axis=mybir.AxisListType.X)
                    r_ = ssb.tile([N, 1], F32, tag="r_")
                    nc.vector.reciprocal(r_[:], s_[:])
                    adj_b = ssb.tile([N, N], BF, tag="adj_b")
                    nc.vector.tensor_scalar_mul(adj_b[:], e_sb[:], r_[:])

                    aaa_ps = gps.tile([N, C], F32, tag="aaa_ps")
                    for hh in range(2):
                        nc.tensor.matmul(
                            aaa_ps[:], nodeT_h[hh][:], wemb_h[hh][:],
                            start=(hh == 0), stop=(hh == 1),
                        )
                    aaa_b = ssb.tile([N, C], BF, tag="aaa_b")
                    nc.scalar.copy(aaa_b[:], aaa_ps[:])
                    aaaT_h = []
                    for hh in range(2):
                        aT_ps = gps.tile([CH, N], BF, tag="aT_ps")
                        nc.tensor.transpose(
                            aT_ps[:], aaa_b[:, hh * CH:(hh + 1) * CH],
                            ident[:N, :N],
                        )
                        aT = ssb.tile([CH, N], BF, tag=f"aaaT{hh}")
                        nc.scalar.copy(aT[:], aT_ps[:])
                        aaaT_h.append(aT)
                    supp_ps = gps.tile([N, C], F32, tag="supp_ps")
                    for hh in range(2):
                        nc.tensor.matmul(
                            supp_ps[:], aaaT_h[hh][:], wgcn_h[hh][:],
                            start=(hh == 0), stop=(hh == 1),
                        )
                    supp_b = ssb.tile([N, C], BF, tag="supp_b")
                    nc.scalar.copy(supp_b[:], supp_ps[:])
                    adjT_ps = gps.tile([N, N], BF, tag="adjT_ps")
                    nc.tensor.transpose(adjT_ps[:], adj_b[:], ident[:N, :N])
                    adjT_b = ssb.tile([N, N], BF, tag="adjT_b")
                    nc.scalar.copy(adjT_b[:], adjT_ps[:])
                    outg_ps = gps.tile([N, C], F32, tag="outg_ps")
                    nc.tensor.matmul(
                        outg_ps[:], adjT_b[:], supp_b[:], start=True, stop=True
                    )
                    nc.vector.tensor_add(outgb[:], outg_ps[:], bb_sb[:])
                    # zero-padded per-t copies so residual matmuls contract
                    # P=18 with partition base 0
                    for t in range(T):
                        nc.any.memset(outg_t[t][:], 0.0)
                        nc.sync.dma_start(
                            outg_t[t][K * t:K * (t + 1), :],
                            outgb[K * t:K * (t + 1), :],
                        )
                    # lhsr[r]: [18, 128] block tile with outg rows 3t:3t+3 in
                    # the column range of each t-span, zeros elsewhere
                    for r in range(NCH):
                        L = lhsr[r]
                        for (t, lo, hi, clo) in _spans(r):
                            nc.any.tensor_copy(
                                L[:, lo:hi], outg_t[t][:, clo:clo + (hi - lo)]
                            )

                # ---------------- pass 2: residual ----------------
                with (
                    tc.tile_pool(name="resps", bufs=KNOBS['res_bufs'],
                                 space="PSUM") as rps,
                    tc.tile_pool(name="ressb", bufs=KNOBS['rsb_bufs']) as rsb,
                ):
                    order = [8, 6] + list(range(NSTASH)) + [7]
                    for r in order:
                        buf = chunk_buf(r)
                        if r in (6, 7):
                            nc.sync.dma_start(
                                buf[:], x[128 * r:128 * (r + 1), :]
                            )
                        xq = buf.rearrange("p (q c) -> p q c", q=4)
                        res_sb = rsb.tile([128, HW0], BF, tag="res_sb")
                        for j in range(NR):
                            res = rps.tile([128, RW], F32, tag="res")
                            nc.tensor.matmul(
                                res[:],
                                lhsr[r][:],
                                m56_sb[:, j * RW:(j + 1) * RW],
                                start=True, stop=True,
                            )
                            sl = slice(j * RW, (j + 1) * RW)
                            if hasattr(rcopy_eng, 'tensor_copy'):
                                rcopy_eng.tensor_copy(res_sb[:, sl], res[:])
                            else:
                                rcopy_eng.copy(res_sb[:, sl], res[:])
                        # nearest-upsample == the same 56-res residual added
                        # to each phase plane: 4 full-width step-1 bf16 adds
                        # (DVE 2x packed mode)
                        for q in range(4):
                            add_eng.tensor_add(xq[:, q, :], xq[:, q, :],
                                               res_sb[:])
                        store_eng.dma_start(y[128 * r:128 * (r + 1), :], buf[:])
    return nc


def _host_prep(x, gcn_masks, W_emb, W_gcn, b_gcn):
    x = np.asarray(x)
    gcn_masks = np.asarray(gcn_masks)
    wembT = np.asarray(W_emb).T.astype(BF_NP)
    wgcnv = np.ascontiguousarray(np.asarray(W_gcn)).astype(BF_NP)
    bbv = np.ascontiguousarray(
        np.broadcast_to(np.asarray(b_gcn, np.float32)[None, :], (N, C))
    )
    in_maps = []
    for b in range(B):
        # phase-major layout: [TC, dh, dw, h0, w0] so the 2x2 box-sum and
        # the nearest-upsample residual add are step-1 ops on device
        xb = np.ascontiguousarray(
            np.asarray(x[:, b]).reshape(T * C, H0, 2, W0, 2)
            .transpose(0, 2, 4, 1, 3).reshape(T * C, HW)
        ).astype(BF_NP)
        m = gcn_masks[b].reshape(T, K, HW0).astype(BF_NP)
        m56v = np.ascontiguousarray(m.reshape(N, HW0))
        mTpv = np.ascontiguousarray(
            m.reshape(T, K, NJ, JW).transpose(3, 0, 2, 1).reshape(JW, T * NJ * K)
        )
        in_maps.append({
            "x": xb, "m56": m56v, "mTp": mTpv,
            "wembT": wembT, "wgcn": wgcnv, "bb": bbv,
        })
    return in_maps


_NC_CACHE = {}


def kernel(x, gcn_masks, W_emb, W_gcn, b_gcn):
    from concourse.bass_utils import run_bass_kernel_spmd

    in_maps = _host_prep(x, gcn_masks, W_emb, W_gcn, b_gcn)
    if "nc" not in _NC_CACHE:
        _NC_CACHE["nc"] = build_nc(reps=1)
    nc = _NC_CACHE["nc"]
    res = run_bass_kernel_spmd(nc, in_maps, list(range(B)))
    out = np.empty((T, B, C, H, W), np.float32)
    for b in range(B):
        yb = res.results[b]["y"].astype(np.float32)
        out[:, b] = (
            yb.reshape(T * C, 2, 2, H0, W0).transpose(0, 3, 1, 4, 2)
            .reshape(T, C, H, W)
        )
    return out


# revision 29
# speedup vs baseline: 2.3288x; 2.3288x over previous
"""Trainium2 Bass kernel for nn_Encoder_90469191122997 (gnn_message_passing).

Data-parallel over batch B=8: core b owns batch b end-to-end.

v3: 16-bit traffic + SBUF residency + phase-major layout + no barriers.

x is sent to the device as bf16 (host cast) in PHASE-MAJOR layout
([TC, dh, dw, h0, w0], the four 2x2-nearest phases separated), y returns
bf16 phase-major (host upcast + re-interleave).  The 2e-2 rel-err gate
leaves ~4x margin.

Per core x_b = [T*C, HW] bf16 = 28.9 MB.  Chunks 0..6 (128 rows each)
stay resident in SBUF between the pooling pass and the residual pass;
chunks 7, 8 stream through two quarter-width buffers and are re-read in
pass 2 (re-read DMAs issued before the GCN so DMA never idles).  HBM
traffic/core: 28.9 (read) + 6.4 (re-read) + 28.9 (write) = 64 MB vs
173 MB for the fp32 two-pass baseline.

Phase-major makes the 2x2 box-sum three full-width step-1 bf16 adds
(DVE 2x packed mode) and the nearest-upsample residual adds pure step-1
adds against a PSUM->SBUF copy of the residual (also 2x).  Pooling
contracts the box-summed transpose (PE, 112-col grid, no tail) against
pre-transposed masks; the 18-node GCN runs on-chip in fp32 PSUM with
bf16 operands.  All tile pools live at one scope: no per-rep drain
barriers; x2 and m56 timeshare one buffer (m56 re-loaded per rep).
"""

import numpy as np
import ml_dtypes

import concourse.bass as bass
import concourse.mybir as mybir
import concourse.tile as tile
from concourse.masks import make_identity

T, B, C, H, W = 6, 8, 192, 112, 112
K = 3
H0, W0 = 56, 56
HW = H * W            # 12544
HW0 = H0 * W0         # 3136
QW = HW0 // 4         # 784, quarter width at 56-res (phase-plane cols)
N = T * K             # 18
CH = 96               # c half
NJ = 28               # pooling blocks per chunk (112-col grid, no tail)
JW = 112              # pooling block width
NR = 8                # residual blocks per chunk
RW = HW0 // NR        # 392, residual block width at 56-res
NCH = T * C // 128    # 9 row-chunks of 128 (t,c) rows each
NSTASH = 7            # chunks 0..6 resident in SBUF; 7, 8 streamed+re-read

BF = mybir.dt.bfloat16
F32 = mybir.dt.float32
BF_NP = ml_dtypes.bfloat16


def _spans(r):
    """(t, lo, hi, clo): rows [lo,hi) of chunk r belong to t, starting at
    channel clo.  Chunk boundaries hit t-edges only at offsets 0/64."""
    out = []
    for t in range(T):
        lo = max(128 * r, C * t)
        hi = min(128 * r + 128, C * (t + 1))
        if lo < hi:
            out.append((t, lo - 128 * r, hi - 128 * r, lo - C * t))
    return out


_LAST_CHUNK = {t: (C * (t + 1) - 1) // 128 for t in range(T)}

_MAX_WAITS = 1


def _split_multi_waits(nc):
    """This container's walrus rejects >1 sem wait per instruction ("Too many
    sync wait commands").  Move extra waits onto same-engine NoOps inserted
    immediately before the instruction (per-engine program order preserved)."""
    for bb in nc.main_func.blocks:
        insts = list(bb.instructions)
        if not any(
            i.sync_info and i.sync_info.on_wait
            and len(i.sync_info.on_wait) > _MAX_WAITS
            for i in insts
        ):
            continue
        new = []
        for inst in insts:
            si = inst.sync_info
            if si and si.on_wait and len(si.on_wait) > _MAX_WAITS:
                extra = list(si.on_wait[_MAX_WAITS:])
                del si.on_wait[_MAX_WAITS:]
                while extra:
                    chunk, extra = extra[:_MAX_WAITS], extra[_MAX_WAITS:]
                    nop = mybir.InstNoOp(
                        name=nc.get_next_instruction_name(),
                        engine=inst.engine,
                        bass_nofuse=True,
                        sync_info=mybir.SyncInfo(on_wait=chunk, on_update=[]),
                    )
                    nc.register_instruction(nop, overwrite=True)
                    new.append(nop)
            new.append(inst)
        bb.instructions = new


_orig_drain_and_barrier = tile.TileContext._drain_and_barrier


def _patched_drain_and_barrier(self, tick_clock, wait_clock):
    _orig_drain_and_barrier(self, tick_clock, wait_clock)
    _split_multi_waits(self.nc)


tile.TileContext._drain_and_barrier = _patched_drain_and_barrier


KNOBS = dict(
    copy_eng='scalar',     # engine for PSUM->SBUF transpose-tile copies
    add_eng='vector',      # engine for pass-2 residual adds
    rcopy_eng='scalar',    # engine for pass-2 res PSUM->SBUF copies
    store_eng='scalar',    # engine issuing y store DMAs
    x2T_bufs=3, tr_bufs=2, res_bufs=2, rsb_bufs=2, feat_bufs=2,
    union=True,            # m56 shares the x2 buffer (False: own tile)
)

if __name__ != "__main__":
    import json as _json
    import os as _os
    _ov = _os.environ.get("KERNEL_KNOBS")
    if _ov:
        KNOBS.update(_json.loads(_ov))


def _copy(eng, dst, src):
    if hasattr(eng, 'tensor_copy'):
        eng.tensor_copy(dst, src)
    else:
        eng.copy(dst, src)


def build_nc(reps: int = 1) -> bass.Bass:
    nc = bass.Bass()
    x = nc.dram_tensor("x", [T * C, HW], BF, kind="ExternalInput")
    m56 = nc.dram_tensor("m56", [N, HW0], BF, kind="ExternalInput")
    mTp = nc.dram_tensor("mTp", [JW, T * NJ * K], BF, kind="ExternalInput")
    wembT = nc.dram_tensor("wembT", [C, C], BF, kind="ExternalInput")
    wgcn = nc.dram_tensor("wgcn", [C, C], BF, kind="ExternalInput")
    bb = nc.dram_tensor("bb", [N, C], F32, kind="ExternalInput")
    y = nc.dram_tensor("y", [T * C, HW], BF, kind="ExternalOutput")

    copy_eng = getattr(nc, KNOBS['copy_eng'])
    add_eng = getattr(nc, KNOBS['add_eng'])
    rcopy_eng = getattr(nc, KNOBS['rcopy_eng'])
    store_eng = getattr(nc, KNOBS['store_eng'])

    with tile.TileContext(nc) as tc:
        with (
            tc.tile_pool(name="persist", bufs=1) as pp,
            tc.tile_pool(name="x2Tpool", bufs=KNOBS['x2T_bufs']) as x2Tpool,
            tc.tile_pool(name="smallsb", bufs=1) as ssb,
            tc.tile_pool(name="ressb", bufs=KNOBS['rsb_bufs']) as rsb,
            tc.tile_pool(name="lhsrp", bufs=2) as lhsrp,
            tc.tile_pool(name="trbps", bufs=KNOBS['tr_bufs'],
                         space="PSUM") as trbps,
            tc.tile_pool(name="featps", bufs=KNOBS['feat_bufs'],
                         space="PSUM") as fps,
            tc.tile_pool(name="ntps", bufs=1, space="PSUM") as ntps,
            tc.tile_pool(name="resps", bufs=KNOBS['res_bufs'],
                         space="PSUM") as rps,
            tc.tile_pool(name="gcnps", bufs=1, space="PSUM") as gps,
        ):
            ident = pp.tile([128, 128], BF)
            make_identity(nc, ident)
            ident32 = pp.tile([128, 128], F32, tag="ident32")
            make_identity(nc, ident32)
            mTp_sb = pp.tile([JW, T * NJ * K], BF)
            nc.sync.dma_start(mTp_sb[:], mTp[:])
            wemb_h = []
            wgcn_h = []
            for hh in range(2):
                wt = pp.tile([CH, C], BF, tag=f"wemb{hh}")
                nc.sync.dma_start(wt[:], wembT[hh * CH:(hh + 1) * CH, :])
                wemb_h.append(wt)
                gt = pp.tile([CH, C], BF, tag=f"wgcn{hh}")
                nc.sync.dma_start(gt[:], wgcn[hh * CH:(hh + 1) * CH, :])
                wgcn_h.append(gt)
            bb_sb = pp.tile([N, C], F32)
            nc.sync.dma_start(bb_sb[:], bb[:])

            # resident x chunks, two quarter-width stream buffers, and the
            # x2 / m56 timeshared scratch
            st = [
                pp.tile([128, HW], BF, tag=f"stash{i}", name=f"stash{i}")
                for i in range(NSTASH)
            ]
            hb = [
                pp.tile([128, HW // 4], BF, tag=f"hb{i}", name=f"hb{i}")
                for i in range(2)
            ]
            u = pp.tile([128, HW0], BF, tag="u", name="u")
            x2 = u            # pass-1 box-sum scratch (all 128 partitions)
            if KNOBS['union']:
                m56_sb = u    # pass-2 masks live in partitions 0..18
            else:
                m56_sb = pp.tile([128, HW0], BF, tag="m56s", name="m56s")

            for rep in range(reps):
                nodeT_h = [
                    pp.tile([CH, N], BF, tag=f"nodeT{hh}", name=f"nodeT{hh}")
                    for hh in range(2)
                ]
                outgb = pp.tile([N, C], BF, tag="outgb", name="outgb")

                # ---------------- pass 1: pooling ----------------
                feat_ps = {}

                def do_pair(r, js):
                    """x2T tile [112, 256] for blocks (2js, 2js+1) of chunk r
                    + pooling matmuls.  Pairing halves the copy count."""
                    jj = (2 * js, 2 * js + 1)
                    tr = trbps.tile([JW, 256], BF, tag="trb")
                    for i, j in enumerate(jj):
                        nc.tensor.transpose(
                            tr[:, 128 * i:128 * (i + 1)],
                            x2[:, j * JW:(j + 1) * JW], ident[:],
                        )
                    x2T = x2Tpool.tile([JW, 256], BF, tag="x2T")
                    _copy(copy_eng, x2T[:], tr[:])
                    for i, j in enumerate(jj):
                        for (t, lo, hi, clo) in _spans(r):
                            col = (t * NJ + j) * K
                            nc.tensor.matmul(
                                feat_ps[t][:, clo:clo + (hi - lo)],
                                mTp_sb[:, col:col + K],
                                x2T[:, 128 * i + lo:128 * i + hi],
                                start=(j == 0), stop=(j == NJ - 1),
                                skip_group_check=True,
                            )

                def boxsum(xq, cols):
                    """x2[:, cols] = sum of the 4 phase planes (step-1 bf16
                    adds, DVE 2x packed mode)."""
                    out = x2[:, cols]
                    nc.vector.tensor_add(out, xq[:, 0, :], xq[:, 1, :])
                    nc.vector.tensor_add(out, out, xq[:, 2, :])
                    nc.vector.tensor_add(out, out, xq[:, 3, :])

                def close_feat(r):
                    for (t, lo, hi, clo) in _spans(r):
                        if _LAST_CHUNK[t] != r:
                            continue
                        feat_sb = ssb.tile([K, C], BF, tag="feat_sb")
                        nc.scalar.mul(feat_sb[:], feat_ps.pop(t)[:], 1.0 / HW)
                        for hh in range(2):
                            ntr = ntps.tile([CH, K], BF, tag="ntr")
                            nc.tensor.transpose(
                                ntr[:],
                                feat_sb[:, hh * CH:(hh + 1) * CH],
                                ident[:K, :K],
                            )
                            nc.any.tensor_copy(
                                nodeT_h[hh][:, K * t:K * (t + 1)], ntr[:]
                            )

                def open_feat(r):
                    for (t, lo, hi, clo) in _spans(r):
                        if t not in feat_ps:
                            feat_ps[t] = fps.tile(
                                [K, C], F32, tag="feat_ps", name=f"featps{t}"
                            )

                for r in range(NSTASH):
                    buf = st[r]
                    nc.sync.dma_start(buf[:], x[128 * r:128 * (r + 1), :])
                    xq = buf.rearrange("p (q c) -> p q c", q=4)
                    open_feat(r)
                    # box-sum in two halves so PE transposes of the first
                    # half overlap DVE summing the second
                    for hf in range(2):
                        sl = slice(hf * (HW0 // 2), (hf + 1) * (HW0 // 2))
                        boxsum(xq[:, :, sl], sl)
                        for js in range(hf * (NJ // 4), (hf + 1) * (NJ // 4)):
                            do_pair(r, js)
                    close_feat(r)

                # chunks 7, 8 stream through the quarter buffers: quarter qt
                # of chunk r holds phase-plane cols [qt*QW, (qt+1)*QW)
                def qslice(r, qt):
                    xv = x[128 * r:128 * (r + 1), :].rearrange(
                        "p (q c) -> p q c", q=4)
                    return xv[:, :, qt * QW:(qt + 1) * QW]

                for r in (7, 8):
                    open_feat(r)
                    for qt in range(4):
                        buf = hb[qt % 2]
                        nc.sync.dma_start(
                            buf.rearrange("p (q c) -> p q c", q=4)[:],
                            qslice(r, qt),
                        )
                        sl = slice(qt * QW, (qt + 1) * QW)
                        boxsum(buf.rearrange("p (q c) -> p q c", q=4), sl)
                        for js in range(qt * (NJ // 8), (qt + 1) * (NJ // 8)):
                            do_pair(r, js)
                    close_feat(r)

                # pass-2 data for chunks 7, 8 comes back via the same two
                # quarter buffers.  Issue the first two re-reads now (they
                # land during the GCN); the rest are issued in pass 2 as
                # each slot's previous quarter is consumed.
                QQ = [(r, qt) for r in (7, 8) for qt in range(4)]

                def reread(i):
                    r, qt = QQ[i]
                    nc.sync.dma_start(
                        hb[qt % 2].rearrange("p (q c) -> p q c", q=4)[:],
                        qslice(r, qt),
                    )

                reread(0)
                reread(1)
                # m56 shares u with x2: load after pass 1's last transpose
                nc.sync.dma_start(m56_sb[:N, :], m56[:])

                # ---------------- GCN on [18, 192] ----------------
                # one PSUM bank: every tile is a slice of the single 'g' tag
                def gtile():
                    return gps.tile([128, C], F32, tag="g", name="g")

                adjL = gtile()
                for hh in range(2):
                    nc.tensor.matmul(
                        adjL[:N, :N], nodeT_h[hh][:], nodeT_h[hh][:],
                        start=(hh == 0), stop=(hh == 1),
                    )
                mx = ssb.tile([N, 1], F32, tag="mx")
                nc.vector.reduce_max(mx[:], adjL[:N, :N],
                                     axis=mybir.AxisListType.X)
                nmx = ssb.tile([N, 1], F32, tag="nmx")
                nc.vector.tensor_scalar_mul(nmx[:], mx[:], -1.0)
                e_sb = ssb.tile([N, N], F32, tag="e_sb")
                nc.scalar.activation(
                    e_sb[:], adjL[:N, :N], mybir.ActivationFunctionType.Exp,
                    bias=nmx[:], scale=1.0,
                )
                s_ = ssb.tile([N, 1], F32, tag="s_")
                nc.vector.reduce_sum(s_[:], e_sb[:], axis=mybir.AxisListType.X)
                r_ = ssb.tile([N, 1], F32, tag="r_")
                nc.vector.reciprocal(r_[:], s_[:])
                adj_f = ssb.tile([N, N], F32, tag="adj_f")
                nc.vector.tensor_scalar_mul(adj_f[:], e_sb[:], r_[:])

                aaa_ps = gtile()
                for hh in range(2):
                    nc.tensor.matmul(
                        aaa_ps[:N, :], nodeT_h[hh][:], wemb_h[hh][:],
                        start=(hh == 0), stop=(hh == 1),
                    )
                aaa_f = ssb.tile([N, C], F32, tag="aaa_f")
                nc.scalar.copy(aaa_f[:], aaa_ps[:N, :])
                aaaT_h = []
                for hh in range(2):
                    aT_ps = gtile()
                    nc.tensor.transpose(
                        aT_ps[:CH, :N], aaa_f[:, hh * CH:(hh + 1) * CH],
                        ident32[:N, :N],
                    )
                    aT = ssb.tile([CH, N], BF, tag=f"aaaT{hh}")
                    nc.scalar.copy(aT[:], aT_ps[:CH, :N])
                    aaaT_h.append(aT)
                supp_ps = gtile()
                for hh in range(2):
                    nc.tensor.matmul(
                        supp_ps[:N, :], aaaT_h[hh][:], wgcn_h[hh][:],
                        start=(hh == 0), stop=(hh == 1),
                    )
                supp_b = ssb.tile([N, C], BF, tag="supp_b")
                nc.scalar.copy(supp_b[:], supp_ps[:N, :])
                adjT_ps = gtile()
                nc.tensor.transpose(adjT_ps[:N, :N], adj_f[:],
                                    ident32[:N, :N])
                adjT_b = ssb.tile([N, N], BF, tag="adjT_b")
                nc.scalar.copy(adjT_b[:], adjT_ps[:N, :N])
                outg_ps = gtile()
                nc.tensor.matmul(
                    outg_ps[:N, :], adjT_b[:], supp_b[:], start=True, stop=True
                )
                nc.vector.tensor_add(outgb[:], outg_ps[:N, :], bb_sb[:])

                # ---------------- pass 2: residual ----------------
                def make_lhsr(r):
                    """[18, 128] tile: outg rows 3t:3t+3 in the column range
                    of each t-span, zeros elsewhere.  Small SBUF->SBUF DMAs
                    go on the store queue to stay clear of the re-reads."""
                    L = lhsrp.tile([N, 128], BF, tag="lhsr")
                    nc.vector.memset(L[:], 0.0)
                    for (t, lo, hi, clo) in _spans(r):
                        store_eng.dma_start(
                            L[K * t:K * (t + 1), lo:hi],
                            outgb[K * t:K * (t + 1), clo:clo + (hi - lo)],
                        )
                    return L

                def res_half(L, h):
                    """Residual for hw0 cols [h*HW0/2, (h+1)*HW0/2) ->
                    bf16 SBUF tile (copies feed the 2x-mode adds)."""
                    rs = rsb.tile([128, HW0 // 2], BF, tag="rs")
                    for j in range(NR // 2):
                        res = rps.tile([128, RW], F32, tag="res")
                        nc.tensor.matmul(
                            res[:],
                            L[:],
                            m56_sb[:N, (h * NR // 2 + j) * RW:
                                   (h * NR // 2 + j + 1) * RW],
                            start=True, stop=True,
                        )
                        _copy(rcopy_eng, rs[:, j * RW:(j + 1) * RW], res[:])
                    return rs

                # streamed chunks first: their re-reads are already in flight
                for r in (7, 8):
                    L = make_lhsr(r)
                    for h in range(2):
                        rs = res_half(L, h)
                        for qt in (2 * h, 2 * h + 1):
                            buf = hb[qt % 2]
                            xq = buf.rearrange("p (q c) -> p q c", q=4)
                            rsl = rs[:, (qt % 2) * QW:(qt % 2 + 1) * QW]
                            for q in range(4):
                                add_eng.tensor_add(xq[:, q, :], xq[:, q, :],
                                                   rsl)
                            yv = y[128 * r:128 * (r + 1), :].rearrange(
                                "p (q c) -> p q c", q=4)
                            store_eng.dma_start(
                                yv[:, :, qt * QW:(qt + 1) * QW], xq[:],
                            )
                            nxt = QQ.index((r, qt)) + 2
                            if nxt < len(QQ):
                                reread(nxt)
                for r in range(NSTASH):
                    buf = st[r]
                    L = make_lhsr(r)
                    xq = buf.rearrange("p (q c) -> p q c", q=4)
                    for h in range(2):
                        rs = res_half(L, h)
                        sl = slice(h * (HW0 // 2), (h + 1) * (HW0 // 2))
                        for q in range(4):
                            add_eng.tensor_add(xq[:, q, sl], xq[:, q, sl],
                                               rs[:])
                    store_eng.dma_start(y[128 * r:128 * (r + 1), :], buf[:])
    return nc


def _host_prep(x, gcn_masks, W_emb, W_gcn, b_gcn):
    x = np.asarray(x)
    gcn_masks = np.asarray(gcn_masks)
    wembT = np.asarray(W_emb).T.astype(BF_NP)
    wgcnv = np.ascontiguousarray(np.asarray(W_gcn)).astype(BF_NP)
    bbv = np.ascontiguousarray(
        np.broadcast_to(np.asarray(b_gcn, np.float32)[None, :], (N, C))
    )
    in_maps = []
    for b in range(B):
        # phase-major layout: [TC, dh, dw, h0, w0] so the 2x2 box-sum and
        # the nearest-upsample residual add are step-1 ops on device
        xb = np.ascontiguousarray(
            np.asarray(x[:, b]).reshape(T * C, H0, 2, W0, 2)
            .transpose(0, 2, 4, 1, 3).reshape(T * C, HW)
        ).astype(BF_NP)
        m = gcn_masks[b].reshape(T, K, HW0).astype(BF_NP)
        m56v = np.ascontiguousarray(m.reshape(N, HW0))
        mTpv = np.ascontiguousarray(
            m.reshape(T, K, NJ, JW).transpose(3, 0, 2, 1).reshape(JW, T * NJ * K)
        )
        in_maps.append({
            "x": xb, "m56": m56v, "mTp": mTpv,
            "wembT": wembT, "wgcn": wgcnv, "bb": bbv,
        })
    return in_maps


_NC_CACHE = {}


def kernel(x, gcn_masks, W_emb, W_gcn, b_gcn):
    from concourse.bass_utils import run_bass_kernel_spmd

    in_maps = _host_prep(x, gcn_masks, W_emb, W_gcn, b_gcn)
    if "nc" not in _NC_CACHE:
        _NC_CACHE["nc"] = build_nc(reps=1)
    nc = _NC_CACHE["nc"]
    res = run_bass_kernel_spmd(nc, in_maps, list(range(B)))
    out = np.empty((T, B, C, H, W), np.float32)
    for b in range(B):
        yb = res.results[b]["y"].astype(np.float32)
        out[:, b] = (
            yb.reshape(T * C, 2, 2, H0, W0).transpose(0, 3, 1, 4, 2)
            .reshape(T, C, H, W)
        )
    return out
